# revision 3
# baseline (speedup 1.0000x reference)
"""RNN-T Joiner kernel for Trainium2, SPMD over 8 NeuronCores.

Reference computation (per batch b):
    hf = ft[b] @ w1[:, :ENC].T            # [T, J]
    hg = gu[b] @ w1[:, ENC:].T            # [U, J]
    joint = tanh(hf[:, None, :] + hg[None, :, :])   # [T, U, J]
    out[b] = joint @ w2.T                 # [T, U, V]

Sharding: data-parallel over B — each of the 8 cores handles one batch
element, full weights replicated. No collectives.

v2 pipeline (vs v1): the per-u biased tanh is split into
  (a) DVE tensor_scalar adds (per-partition vector bias, 4x perf mode,
      fp16) producing sums[j, u, t], and
  (b) one giant ScalarE Tanh per 8-u block (N=4096; the (N+224)/1.2
      ACT cost makes big instructions ~1.7x cheaper per element than
      the v1 per-(u,jo) biased tanh).
The big fp16 GEMM accumulates into 2-u PSUM tiles (4 banks, ring of 2
= all 8 banks) so the PE can run several u ahead. PSUM evacuation
(fp32->fp16, the single largest engine cost at ~64k elem/partition,
1x on both engines) is split between ScalarE and DVE to balance them.
Output is DMA'd per (to, 8-u block) in 1MB transfers.
"""

import numpy as np

import concourse.bass as bass
import concourse.mybir as mybir
import concourse.tile as tile
from concourse import bacc
from concourse.bass_utils import run_bass_kernel_spmd
from concourse.masks import make_identity

B, T, U = 8, 256, 64
ENC, PRED = 128, 256
J, V = 256, 500
N_CORES = 8
P = 128
f32 = mybir.dt.float32
f16 = mybir.dt.float16

UB = 8             # u-block size (one tanh instruction per block)
NBLK = U // UB     # 8 blocks
# evac engine split: per 2-u PSUM tile, True -> ScalarE, False -> DVE.
# Balanced so ACT(tanh + evac) ~= DVE(adds + evac).
EVAC_ACT_FRAC = 0.5


def _emit(nc, tc, ft, gu, w1, w2, out):
    JO = J // P          # 2 chunks of j
    TO = T // P          # 2 chunks of t
    with (
        tc.tile_pool(name="const", bufs=1) as const,
        tc.tile_pool(name="sums", bufs=3) as spool,
        tc.tile_pool(name="joint", bufs=3) as jpool,
        tc.tile_pool(name="ot", bufs=3) as opool,
    ):
        ident = const.tile([P, P], f32)
        make_identity(nc, ident)

        # ---- loads: one contiguous multi-KB DRAM run per partition ----
        # Row r lands on partition r//G at sub-index r%G, so each
        # partition line is a single contiguous run (fewer DMA
        # descriptors); the transpose copies below write stride-G to
        # restore true column order.
        ft_sb = const.tile([P, TO, ENC], f32)         # [p, to, e]: t = 2p+to
        nc.sync.dma_start(ft_sb[:], ft.ap().rearrange("(p to) e -> p to e", p=P))
        gu_sb = const.tile([P, PRED], f32)            # [u (64 used), p]
        nc.scalar.dma_start(gu_sb[:U, :], gu.ap())
        w1_sb = const.tile([P, JO, ENC + PRED], f32)  # [p, jo2, e]: j = 2p+jo2
        nc.scalar.dma_start(w1_sb[:], w1.ap().rearrange("(p jo) e -> p jo e", p=P))
        w2_sb = const.tile([P, 4, J], f32)            # [p (125 used), vo, j]: v = 4p+vo
        nc.sync.dma_start(
            w2_sb[:125, :, :], w2.ap().rearrange("(p vo) j -> p vo j", p=125)
        )

        # ---- PE transposes into contraction-major layouts ----
        pst_cm = tc.tile_pool(name="pst", bufs=4, space="PSUM")
        pst = pst_cm.__enter__()
        psg_cm = tc.tile_pool(name="psg", bufs=2, space="PSUM")
        psg = psg_cm.__enter__()

        # ftT[e, i, to]: flat free = t = 2i+to
        ftT = const.tile([P, P, TO], f32)
        for to in range(TO):
            pt = pst.tile([P, P], f32, tag="pt")
            nc.tensor.transpose(pt[:], ft_sb[:, to, :], ident[:])
            if to == 0:
                nc.vector.tensor_copy(ftT[:, :, to], pt[:])
            else:
                nc.scalar.copy(ftT[:, :, to], pt[:])

        # guT[p, pc, u]
        guT = const.tile([P, PRED // P, U], f32)
        for pc in range(PRED // P):
            pt = pst.tile([P, P], f32, tag="pt")
            nc.tensor.transpose(pt[:], gu_sb[:, pc * P : (pc + 1) * P], ident[:])
            if pc == 0:
                nc.vector.tensor_copy(guT[:, pc, :], pt[:, :U])
            else:
                nc.scalar.copy(guT[:, pc, :], pt[:, :U])

        # w1T[k, kc, i, jo2]: flat free = j = 2i+jo2
        w1T = const.tile([P, 3, P, JO], f32)
        for jo2 in range(JO):
            for kc in range(3):
                pt = pst.tile([P, P], f32, tag="pt")
                nc.tensor.transpose(
                    pt[:], w1_sb[:, jo2, kc * P : (kc + 1) * P], ident[:]
                )
                if jo2 == 0:
                    nc.vector.tensor_copy(w1T[:, kc, :, jo2], pt[:])
                else:
                    nc.scalar.copy(w1T[:, kc, :, jo2], pt[:])

        # ---- first GEMMs (fp32): hf[j, t], hg[j, u] -> fp16 SBUF ----
        # hf_sb[p, jo, t]: j = 128*jo + p
        hf_sb = const.tile([P, JO, T], f16)
        for jo in range(JO):
            ph = psg.tile([P, T], f32, tag="ph")
            nc.tensor.matmul(
                ph[:],
                w1T[:, 0, 64 * jo : 64 * (jo + 1), :],
                ftT[:],
                start=True,
                stop=True,
            )
            if jo == 0:
                nc.vector.tensor_copy(hf_sb[:, jo, :], ph[:])
            else:
                nc.scalar.copy(hf_sb[:, jo, :], ph[:])

        # hgT[p, jo, u]: j = 128*jo + p (f32: DVE tensor_scalar requires
        # an fp32 per-partition scalar operand)
        hgT = const.tile([P, JO, U], f32)
        for jo in range(JO):
            ph = psg.tile([P, U], f32, tag="phg")
            for pc in range(PRED // P):
                nc.tensor.matmul(
                    ph[:],
                    w1T[:, 1 + pc, 64 * jo : 64 * (jo + 1), :],
                    guT[:, pc, :],
                    start=(pc == 0),
                    stop=(pc == 1),
                )
            if jo == 0:
                nc.vector.tensor_copy(hgT[:, jo, :], ph[:])
            else:
                nc.scalar.copy(hgT[:, jo, :], ph[:])

        # w2T[j, jo, i, vo]: flat free = v = 4i+vo (fp16 for the big
        # GEMM) — transposed last; only needed once the first big matmul
        # issues.
        w2T = const.tile([P, JO, 125, 4], f16)
        for vo in range(4):
            for jo in range(JO):
                pt = pst.tile([P, P], f32, tag="pt")
                nc.tensor.transpose(
                    pt[:], w2_sb[:, vo, jo * P : (jo + 1) * P], ident[:]
                )
                nc.vector.tensor_copy(w2T[:, jo, :, vo], pt[:, :125])

        psg_cm.__exit__(None, None, None)
        pst_cm.__exit__(None, None, None)

        # ---- main loop over 8-u blocks ----
        pso_cm = tc.tile_pool(name="pso", bufs=2, space="PSUM")
        pso = pso_cm.__enter__()

        n_tiles = U // 2            # 32 2-u PSUM tiles
        n_act = int(round(n_tiles * EVAC_ACT_FRAC))
        # spread ACT-evac tiles evenly
        act_evac = set()
        if n_act > 0:
            step = n_tiles / n_act
            act_evac = {int(i * step) for i in range(n_act)}

        for blk in range(NBLK):
            u0 = blk * UB
            # (a) biased sums on DVE: sums[p, uu, jo, t] fp16
            sums = spool.tile([P, UB, JO, T], f16, tag="sums")
            for uu in range(UB):
                for jo in range(JO):
                    nc.vector.tensor_scalar_add(
                        sums[:, uu, jo, :],
                        hf_sb[:, jo, :],
                        hgT[:, jo, u0 + uu : u0 + uu + 1],
                    )
            # (b) one giant tanh on ScalarE for the whole block
            joint = jpool.tile([P, UB, JO, T], f16, tag="joint")
            nc.scalar.activation(
                joint[:],
                sums[:],
                mybir.ActivationFunctionType.Tanh,
            )
            # (c) big GEMM into 2-u PSUM tiles; (d) evac; (e) DMA out
            ot = opool.tile([P, UB, TO, V], f16, tag="ot")
            for pair in range(UB // 2):
                po = pso.tile([P, 2, TO, 512], f32, tag="po")
                for uu2 in range(2):
                    uu = pair * 2 + uu2
                    for to in range(TO):
                        for jo in range(JO):
                            nc.tensor.matmul(
                                po[:, uu2, to, 0:V],
                                joint[:, uu, jo, to * P : (to + 1) * P],
                                w2T[:, jo, :, :],
                                start=(jo == 0),
                                stop=(jo == JO - 1),
                            )
                tile_idx = blk * (UB // 2) + pair
                dst = ot[:, pair * 2 : pair * 2 + 2, :, :]
                src = po[:, :, :, 0:V]
                if tile_idx in act_evac:
                    nc.scalar.copy(dst, src)
                else:
                    nc.vector.tensor_copy(dst, src)
            for to in range(TO):
                nc.sync.dma_start(
                    out.ap()[to * P : (to + 1) * P, u0 : u0 + UB, :],
                    ot[:, :, to, :],
                )
        pso_cm.__exit__(None, None, None)


_NC_CACHE = None


def _build():
    global _NC_CACHE
    if _NC_CACHE is not None:
        return _NC_CACHE
    nc = bacc.Bacc("TRN2", target_bir_lowering=False, debug=False)
    ft = nc.dram_tensor("ft", [T, ENC], f32, kind="ExternalInput")
    gu = nc.dram_tensor("gu", [U, PRED], f32, kind="ExternalInput")
    w1 = nc.dram_tensor("w1", [J, ENC + PRED], f32, kind="ExternalInput")
    w2 = nc.dram_tensor("w2", [V, J], f32, kind="ExternalInput")
    out = nc.dram_tensor("out", [T, U, V], f16, kind="ExternalOutput")
    with tile.TileContext(nc) as tc:
        _emit(nc, tc, ft, gu, w1, w2, out)
    nc.compile()
    _NC_CACHE = nc
    return nc


def run(ft, gu, w1, w2, trace=False):
    """Run the SPMD kernel; returns (output [B,T,U,V], BassKernelResults)."""
    nc = _build()
    w1c = np.ascontiguousarray(w1, dtype=np.float32)
    w2c = np.ascontiguousarray(w2, dtype=np.float32)
    in_maps = [
        {
            "ft": np.ascontiguousarray(ft[b], dtype=np.float32),
            "gu": np.ascontiguousarray(gu[b], dtype=np.float32),
            "w1": w1c,
            "w2": w2c,
        }
        for b in range(B)
    ]
    res = run_bass_kernel_spmd(
        nc, in_maps, core_ids=list(range(N_CORES)), trace=trace
    )
    out = np.stack(
        [res.results[c]["out"].astype(np.float32) for c in range(N_CORES)], axis=0
    )
    return out, res


def kernel(ft, gu, w1, w2):
    out, _ = run(ft, gu, w1, w2, trace=False)
    return out


# revision 4
# speedup vs baseline: 1.2247x; 1.2247x over previous
"""RNN-T Joiner kernel for Trainium2, SPMD over 8 NeuronCores.

Reference computation (per batch b):
    hf = ft[b] @ w1[:, :ENC].T            # [T, J]
    hg = gu[b] @ w1[:, ENC:].T            # [U, J]
    joint = tanh(hf[:, None, :] + hg[None, :, :])   # [T, U, J]
    out[b] = joint @ w2.T                 # [T, U, V]

Sharding: data-parallel over B — each of the 8 cores handles one batch
element, full weights replicated. No collectives.

v2 pipeline (vs v1): the per-u biased tanh is split into
  (a) DVE tensor_scalar adds (per-partition vector bias, 4x perf mode,
      fp16) producing sums[j, u, t], and
  (b) one giant ScalarE Tanh per 8-u block (N=4096; the (N+224)/1.2
      ACT cost makes big instructions ~1.7x cheaper per element than
      the v1 per-(u,jo) biased tanh).
The big fp16 GEMM accumulates into 2-u PSUM tiles (4 banks, ring of 2
= all 8 banks) so the PE can run several u ahead. PSUM evacuation
(fp32->fp16, the single largest engine cost at ~64k elem/partition,
1x on both engines) is split between ScalarE and DVE to balance them.
Output is DMA'd per (to, 8-u block) in 1MB transfers.
"""

import numpy as np

import concourse.bass as bass
import concourse.mybir as mybir
import concourse.tile as tile
from concourse import bacc
from concourse.bass_utils import run_bass_kernel_spmd
from concourse.masks import make_identity

B, T, U = 8, 256, 64
ENC, PRED = 128, 256
J, V = 256, 500
N_CORES = 8
P = 128
f32 = mybir.dt.float32
f16 = mybir.dt.float16

UB = 8             # u-block size (one tanh instruction per block)
NBLK = U // UB     # 8 blocks
# evac engine split: per 2-u PSUM tile, True -> ScalarE, False -> DVE.
# Balanced so ACT(tanh + evac) ~= DVE(adds + evac).
EVAC_ACT_FRAC = 0.5


def _emit(nc, tc, ft, gu, w1, w2, out):
    JO = J // P          # 2 chunks of j
    TO = T // P          # 2 chunks of t
    with (
        tc.tile_pool(name="const", bufs=1) as const,
        tc.tile_pool(name="sums", bufs=3) as spool,
        tc.tile_pool(name="joint", bufs=3) as jpool,
        tc.tile_pool(name="ot", bufs=3) as opool,
    ):
        ident = const.tile([P, P], f32)
        make_identity(nc, ident)

        # ---- loads: one contiguous multi-KB DRAM run per partition ----
        # Row r lands on partition r//G at sub-index r%G, so each
        # partition line is a single contiguous run (fewer DMA
        # descriptors); the transpose copies below write stride-G to
        # restore true column order.
        ft_sb = const.tile([P, TO, ENC], f32)         # [p, to, e]: t = 2p+to
        nc.sync.dma_start(ft_sb[:], ft.ap().rearrange("(p to) e -> p to e", p=P))
        gu_sb = const.tile([P, PRED], f32)            # [u (64 used), p]
        nc.scalar.dma_start(gu_sb[:U, :], gu.ap())
        w1_sb = const.tile([P, JO, ENC + PRED], f32)  # [p, jo2, e]: j = 2p+jo2
        nc.scalar.dma_start(w1_sb[:], w1.ap().rearrange("(p jo) e -> p jo e", p=P))
        w2_sb = const.tile([P, 4, J], f32)            # [p (125 used), vo, j]: v = 4p+vo
        nc.sync.dma_start(
            w2_sb[:125, :, :], w2.ap().rearrange("(p vo) j -> p vo j", p=125)
        )

        # ---- PE transposes into contraction-major layouts ----
        pst_cm = tc.tile_pool(name="pst", bufs=4, space="PSUM")
        pst = pst_cm.__enter__()
        psg_cm = tc.tile_pool(name="psg", bufs=2, space="PSUM")
        psg = psg_cm.__enter__()

        # ftT[e, i, to]: flat free = t = 2i+to
        ftT = const.tile([P, P, TO], f32)
        for to in range(TO):
            pt = pst.tile([P, P], f32, tag="pt")
            nc.tensor.transpose(pt[:], ft_sb[:, to, :], ident[:])
            if to == 0:
                nc.vector.tensor_copy(ftT[:, :, to], pt[:])
            else:
                nc.scalar.copy(ftT[:, :, to], pt[:])

        # guT[p, pc, u]
        guT = const.tile([P, PRED // P, U], f32)
        for pc in range(PRED // P):
            pt = pst.tile([P, P], f32, tag="pt")
            nc.tensor.transpose(pt[:], gu_sb[:, pc * P : (pc + 1) * P], ident[:])
            if pc == 0:
                nc.vector.tensor_copy(guT[:, pc, :], pt[:, :U])
            else:
                nc.scalar.copy(guT[:, pc, :], pt[:, :U])

        # w1T[k, kc, i, jo2]: flat free = j = 2i+jo2
        w1T = const.tile([P, 3, P, JO], f32)
        for jo2 in range(JO):
            for kc in range(3):
                pt = pst.tile([P, P], f32, tag="pt")
                nc.tensor.transpose(
                    pt[:], w1_sb[:, jo2, kc * P : (kc + 1) * P], ident[:]
                )
                if jo2 == 0:
                    nc.vector.tensor_copy(w1T[:, kc, :, jo2], pt[:])
                else:
                    nc.scalar.copy(w1T[:, kc, :, jo2], pt[:])

        # ---- first GEMMs (fp32): hf[j, t], hg[j, u] -> fp16 SBUF ----
        # hf_sb[p, jo, t]: j = 128*jo + p
        hf_sb = const.tile([P, JO, T], f16)
        for jo in range(JO):
            ph = psg.tile([P, T], f32, tag="ph")
            nc.tensor.matmul(
                ph[:],
                w1T[:, 0, 64 * jo : 64 * (jo + 1), :],
                ftT[:],
                start=True,
                stop=True,
            )
            if jo == 0:
                nc.vector.tensor_copy(hf_sb[:, jo, :], ph[:])
            else:
                nc.scalar.copy(hf_sb[:, jo, :], ph[:])

        # hgT[p, jo, u]: j = 128*jo + p (f32: DVE tensor_scalar requires
        # an fp32 per-partition scalar operand)
        hgT = const.tile([P, JO, U], f32)
        for jo in range(JO):
            ph = psg.tile([P, U], f32, tag="phg")
            for pc in range(PRED // P):
                nc.tensor.matmul(
                    ph[:],
                    w1T[:, 1 + pc, 64 * jo : 64 * (jo + 1), :],
                    guT[:, pc, :],
                    start=(pc == 0),
                    stop=(pc == 1),
                )
            if jo == 0:
                nc.vector.tensor_copy(hgT[:, jo, :], ph[:])
            else:
                nc.scalar.copy(hgT[:, jo, :], ph[:])

        # w2T[j, jo, i, vo]: flat free = v = 4i+vo (fp16 for the big
        # GEMM) — transposed last; only needed once the first big matmul
        # issues.
        w2T = const.tile([P, JO, 125, 4], f16)
        for vo in range(4):
            for jo in range(JO):
                pt = pst.tile([P, P], f32, tag="pt")
                nc.tensor.transpose(
                    pt[:], w2_sb[:, vo, jo * P : (jo + 1) * P], ident[:]
                )
                nc.vector.tensor_copy(w2T[:, jo, :, vo], pt[:, :125])

        psg_cm.__exit__(None, None, None)
        pst_cm.__exit__(None, None, None)

        # ---- main loop over 8-u blocks ----
        pso_cm = tc.tile_pool(name="pso", bufs=2, space="PSUM")
        pso = pso_cm.__enter__()

        n_tiles = U // 2            # 32 2-u PSUM tiles
        n_act = int(round(n_tiles * EVAC_ACT_FRAC))
        # spread ACT-evac tiles evenly
        act_evac = set()
        if n_act > 0:
            step = n_tiles / n_act
            act_evac = {int(i * step) for i in range(n_act)}

        for blk in range(NBLK):
            u0 = blk * UB
            # (a) biased sums on DVE: sums[p, uu, jo, t] fp16
            sums = spool.tile([P, UB, JO, T], f16, tag="sums")
            for uu in range(UB):
                for jo in range(JO):
                    nc.vector.tensor_scalar_add(
                        sums[:, uu, jo, :],
                        hf_sb[:, jo, :],
                        hgT[:, jo, u0 + uu : u0 + uu + 1],
                    )
            # (b) tanh on ScalarE: one giant instruction per block.
            # Block 0 is split into 2-u chunks so the first matmuls can
            # issue ~2.5us earlier during pipeline ramp.
            joint = jpool.tile([P, UB, JO, T], f16, tag="joint")
            if blk == 0:
                for pair in range(UB // 2):
                    nc.scalar.activation(
                        joint[:, pair * 2 : pair * 2 + 2, :, :],
                        sums[:, pair * 2 : pair * 2 + 2, :, :],
                        mybir.ActivationFunctionType.Tanh,
                    )
            else:
                nc.scalar.activation(
                    joint[:],
                    sums[:],
                    mybir.ActivationFunctionType.Tanh,
                )
            # (c) big GEMM into 2-u PSUM tiles; (d) evac; (e) DMA out.
            # po layout [to, uu2, 512] matches ot's [to, u, v] order so the
            # evac AP streams in destination order and each output DMA is
            # one contiguous 8KB run per partition.
            ot = opool.tile([P, TO, UB, V], f16, tag="ot")
            for pair in range(UB // 2):
                po = pso.tile([P, TO, 2, 512], f32, tag="po")
                for uu2 in range(2):
                    uu = pair * 2 + uu2
                    for to in range(TO):
                        for jo in range(JO):
                            nc.tensor.matmul(
                                po[:, to, uu2, 0:V],
                                joint[:, uu, jo, to * P : (to + 1) * P],
                                w2T[:, jo, :, :],
                                start=(jo == 0),
                                stop=(jo == JO - 1),
                            )
                tile_idx = blk * (UB // 2) + pair
                dst = ot[:, :, pair * 2 : pair * 2 + 2, :]
                src = po[:, :, :, 0:V]
                if tile_idx in act_evac:
                    nc.scalar.copy(dst, src)
                else:
                    nc.vector.tensor_copy(dst, src)
            for to in range(TO):
                q = nc.sync if to == 0 else nc.gpsimd
                q.dma_start(
                    out.ap()[to * P : (to + 1) * P, u0 : u0 + UB, :],
                    ot[:, to, :, :],
                )
        pso_cm.__exit__(None, None, None)


_NC_CACHE = None


def _build():
    global _NC_CACHE
    if _NC_CACHE is not None:
        return _NC_CACHE
    nc = bacc.Bacc("TRN2", target_bir_lowering=False, debug=False)
    ft = nc.dram_tensor("ft", [T, ENC], f32, kind="ExternalInput")
    gu = nc.dram_tensor("gu", [U, PRED], f32, kind="ExternalInput")
    w1 = nc.dram_tensor("w1", [J, ENC + PRED], f32, kind="ExternalInput")
    w2 = nc.dram_tensor("w2", [V, J], f32, kind="ExternalInput")
    out = nc.dram_tensor("out", [T, U, V], f16, kind="ExternalOutput")
    with tile.TileContext(nc) as tc:
        _emit(nc, tc, ft, gu, w1, w2, out)
    nc.compile()
    _NC_CACHE = nc
    return nc


def run(ft, gu, w1, w2, trace=False):
    """Run the SPMD kernel; returns (output [B,T,U,V], BassKernelResults)."""
    nc = _build()
    w1c = np.ascontiguousarray(w1, dtype=np.float32)
    w2c = np.ascontiguousarray(w2, dtype=np.float32)
    in_maps = [
        {
            "ft": np.ascontiguousarray(ft[b], dtype=np.float32),
            "gu": np.ascontiguousarray(gu[b], dtype=np.float32),
            "w1": w1c,
            "w2": w2c,
        }
        for b in range(B)
    ]
    res = run_bass_kernel_spmd(
        nc, in_maps, core_ids=list(range(N_CORES)), trace=trace
    )
    out = np.stack(
        [res.results[c]["out"].astype(np.float32) for c in range(N_CORES)], axis=0
    )
    return out, res


def kernel(ft, gu, w1, w2):
    out, _ = run(ft, gu, w1, w2, trace=False)
    return out
